# revision 3
# baseline (speedup 1.0000x reference)
"""GAT layer (nn_GATLayerAdj) Trainium2 Bass kernel, 8-core SPMD.

Reference computation (N=1024, di=do=64):
    a[i,j]  = x[j]@w_src + x[i]@w_tgt + bw        (attention logits)
    att     = softmax_j(where(adj>0, a, -1e16))
    y[i,j,:]= relu(x[j]@WfS.T + x[i]@WfT.T + bf)
    o[i,:]  = sum_j att[i,j] * y[i,j,:]

Key factorization: e[i,j] = exp(a[i,j])*M[i,j] with M = (adj>0) splits as
exp(atgt[i]+bw) * exp(asrc[j]) * M[i,j]; the row factor cancels in the
softmax, so att[i,j] = g[j]M[i,j] / sum_j g[j]M[i,j] with g = exp(asrc).
The device needs NO exp / softmax / transposes: the host uploads
e'^T[j,i] = g[j]*M[i,j] (transposed, PE-stationary-ready) and
r_t[i] = 1/sum_j e'^T[j,i] (same O(N^2) prep class as the old adjm
mask); all O(N^2 d) work runs on device.

Sharding: target-node dim i split across 8 cores (128 target rows each).

Per-core schedule (source dim j on partitions):
  1. DMAs split across both HWDGE queues (sync + act), ordered by
     need-time; urep[j,(i,d)] = u broadcast across partitions via
     stride-0 DMA reads (2MB total), sliced fine at the start so the
     first adds start early.
  2. HALF-major build: pass h processes free columns [4096h, 4096h+4096)
     of ALL chunks. Z = ys_bcast + urep on DVE (tensor_tensor, 2x bf16);
     relu split DVE (tensor_scalar_max, 4x) / ACT per a balance table.
     D-chunk relus write two half tiles so the reduce matmuls of the
     first half overlap the second relu.
  3. T_acc += e'^T chunk matmuls; pass h accumulates into its own bank
     set (4 banks per pass, partitions 64h..64h+63 live) so pass-0
     banks evacuate + DMA out during pass 1, leaving only pass-1's
     4 x [64,512] evacuations in the tail.

Numerics: bf16 inputs to the adds/matmuls, fp32 accumulation, bf16
output (host upcasts).
"""

from contextlib import ExitStack

import numpy as np
import ml_dtypes

import concourse.bass as bass
import concourse.tile as tile
from concourse import bacc, mybir
from concourse.bass_utils import run_bass_kernel_spmd

# Lighter TileContext exit: stock emits drain + full butterfly barrier +
# sem clears + second butterfly (~11us). Engines already sync at program
# end; keep the drain (output DMA completion), a sem-only rendezvous
# before the clears, and drop the trailing barrier.
import concourse.tile as _tile_mod

if not getattr(_tile_mod, "_exit_trimmed", False):
    def _drain_and_barrier_trim(self, tick_clock, wait_clock):
        from concourse.tile import ScopedClock
        nc = self.nc
        drain_inst = nc.sync.drain()
        wait_clock.add_sem_waits(
            drain_inst.ins, ScopedClock({None: tick_clock.global_clock})
        )
        exit_sem = nc.alloc_semaphore("exit_rdv")
        for eng in (nc.sync, nc.tensor, nc.vector, nc.scalar):
            eng.nop(nofuse=True).then_inc(exit_sem, 1)
        nc.gpsimd.wait_ge(exit_sem, 4)
        assert self.sems is not None
        popped = nc._tile_sem_poison_stack.pop()
        assert popped is self._sem_poison
        nc.clear_and_free_semaphores(list(self.sems.allocated().values()))
        nc.gpsimd.sem_clear(range(exit_sem.num, exit_sem.num + 1))

    _tile_mod.TileContext._drain_and_barrier = _drain_and_barrier_trim
    _tile_mod._exit_trimmed = True

N = 1024
DI = 64
DO = 64
N_CORES = 8
ROWS = N // N_CORES          # 128 target rows per core
NCHUNK = N // 128            # 8 j-chunks
F_FULL = ROWS * DO           # 8192 free size of (i, d)
HALF = F_FULL // 2           # 4096
QUART = F_FULL // 4          # 2048

f32 = mybir.dt.float32
bf16 = mybir.dt.bfloat16
AF = mybir.ActivationFunctionType
ALU = mybir.AluOpType

# Per-pass chunk emission order and relu-engine split. 'A' = one ACT
# [128,4096] relu, 'D' = two DVE tensor_scalar_max [128,2048] (4x).
# A-chunk TTs must arrive at ACT every <=3.7us or ACT starves, so D
# chunks are interleaved between A chunks. Last chunk of pass 1 is D
# so the tail chain (relu -> final matmuls -> evac -> out) is short.
# Balance: DVE = 16x2.28 + 5x1.36 ~= 43.3; ACT = 11x3.69 ~= 40.6us.
PASS_ORDER = [
    [(0, "A"), (1, "D"), (2, "A"), (4, "A"), (3, "D"), (6, "A"),
     (5, "A"), (7, "A")],
    [(2, "A"), (0, "D"), (3, "A"), (1, "A"), (4, "D"), (5, "A"),
     (6, "A"), (7, "D")],
]

_CACHE = {}


def _build_program():
    nc = bacc.Bacc("TRN2", target_bir_lowering=False, debug=False,
                   num_devices=N_CORES)

    # ---- DRAM I/O ----
    ysjp_d = nc.dram_tensor("ysjp", [128, NCHUNK * DO], bf16,
                            kind="ExternalInput").ap()
    etp_d = nc.dram_tensor("etp", [128, N], bf16,
                           kind="ExternalInput").ap()
    rinv_d = nc.dram_tensor("rinv", [128, 1], f32, kind="ExternalInput").ap()
    uflat_d = nc.dram_tensor("uflat", [F_FULL], bf16, kind="ExternalInput").ap()
    o_d = nc.dram_tensor("o", [128, 2048], bf16, kind="ExternalOutput").ap()

    with tile.TileContext(nc) as tc, ExitStack() as ctx:
        cons = ctx.enter_context(tc.tile_pool(name="cons", bufs=1))
        zp = ctx.enter_context(tc.tile_pool(name="zp", bufs=4))
        rp = ctx.enter_context(tc.tile_pool(name="rp", bufs=4))
        accp = ctx.enter_context(tc.tile_pool(name="accp", bufs=1, space="PSUM"))

        ys_jp = cons.tile([128, NCHUNK * DO], bf16)
        urep = cons.tile([128, F_FULL], bf16)
        etp = cons.tile([128, N], bf16)
        r_t = cons.tile([ROWS, 1], f32)

        def bcast(eng, c0, c1):
            src = uflat_d[c0:c1]
            bsrc = bass.AP(tensor=src.tensor, offset=src.offset,
                           ap=[[0, 128]] + [list(d) for d in src.ap])
            eng.dma_start(out=urep[:, c0:c1], in_=bsrc)

        # ---- DMAs on both HWDGE queues, ordered by need-time.
        # sync: first-build data fine-sliced, then the rest of pass 0.
        # act:  etp + r_t (reduce side), then pass-1 broadcast halves.
        nc.sync.dma_start(ys_jp[:, 0:DO], ysjp_d[:, 0:DO])
        bcast(nc.sync, 0, 512)
        bcast(nc.sync, 512, 1024)
        nc.sync.dma_start(ys_jp[:, DO:], ysjp_d[:, DO:])
        bcast(nc.sync, 1024, 2048)
        bcast(nc.sync, 2048, 4096)
        nc.scalar.dma_start(etp[:], etp_d[:, :])
        nc.scalar.dma_start(r_t[:], rinv_d[:, :])
        bcast(nc.scalar, 4096, 6144)
        bcast(nc.scalar, 6144, 8192)

        et_all = etp[:, 0:N]

        # pass-separated PSUM banks: pass h writes partitions 64h..64h+63
        # of its own 4-bank set, so pass-0 results evacuate + stream out
        # while pass 1 is still accumulating.
        t_accs = [[accp.tile([128, 512], f32, tag=f"acc{h}{n2}",
                             name=f"t_acc{h}{n2}")
                   for n2 in range(4)] for h in range(2)]
        t_sb = cons.tile([128, 2048], bf16)

        def emit_build(h, c, eng):
            z = zp.tile([128, HALF], bf16, name="z")
            ys_c = ys_jp[:, DO * c:DO * (c + 1)]
            # chunk 0 of pass 0: staged sub-adds so the first one only
            # waits for the first 128KB broadcast slice
            subs = (512, 512, 1024, 2048) if (h, c) == (0, 0) else (HALF,)
            pos = 0
            for step in subs:
                sl = slice(HALF * h + pos, HALF * h + pos + step)
                zl = slice(pos, pos + step)
                ys_b = ys_c.rearrange("p d -> p () d").broadcast_to(
                    (128, step // DO, DO))
                zv = z[:, zl].rearrange("p (i d) -> p i d", i=step // DO)
                uv = urep[:, sl].rearrange("p (i d) -> p i d", i=step // DO)
                nc.vector.tensor_tensor(zv, ys_b, uv, ALU.add)
                pos += step
            if eng == "D":
                # two half tiles: the first half's reduce matmuls can
                # start while the second relu runs
                r0 = rp.tile([128, QUART], bf16, name="r0")
                r1 = rp.tile([128, QUART], bf16, name="r1")
                nc.vector.tensor_scalar_max(r0[:], z[:, 0:QUART], 0.0)
                nc.vector.tensor_scalar_max(r1[:], z[:, QUART:], 0.0)
                return (r0, r1)
            r0 = rp.tile([128, HALF], bf16, name="rA")
            nc.scalar.activation(r0[:], z[:], AF.Relu)
            return (r0,)

        def emit_reduce(h, c, parts, first, last):
            for bq in range(2):
                b = 2 * h + bq
                if len(parts) == 2:
                    src = parts[bq]
                    base = 0
                else:
                    src = parts[0]
                    base = 2048 * bq
                for n2 in range(4):
                    nc.tensor.matmul(
                        t_accs[h][n2][32 * b:32 * (b + 1), :],
                        et_all[:, 128 * c + 32 * b:128 * c + 32 * (b + 1)],
                        src[:, base + 512 * n2:base + 512 * (n2 + 1)],
                        start=first,
                        stop=last,
                        skip_group_check=True,
                        tile_position=(0, 32 * b),
                    )
            if last:
                # this pass's accumulation done: four scaled [64,512]
                # evacuations (scale=1/s', DVE/ACT alternating) + output
                # DMAs on the act HWDGE queue. Pass 0's run mid-kernel.
                rows = slice(64 * h, 64 * (h + 1))
                for n2 in range(4):
                    osl = slice(512 * n2, 512 * (n2 + 1))
                    if n2 % 2 == 0:
                        nc.vector.tensor_scalar_mul(t_sb[rows, osl],
                                                    t_accs[h][n2][rows, :],
                                                    r_t[rows])
                    else:
                        nc.scalar.activation(t_sb[rows, osl],
                                             t_accs[h][n2][rows, :],
                                             AF.Copy, bias=0.0,
                                             scale=r_t[rows])
                    nc.scalar.dma_start(out=o_d[rows, osl],
                                        in_=t_sb[rows, osl])

        for h in range(2):
            order = PASS_ORDER[h]
            pend = None
            for k in range(len(order) + 1):
                if k < len(order):
                    c, eng = order[k]
                    built = (c, emit_build(h, c, eng))
                if k >= 1:
                    pc, pr = pend
                    emit_reduce(h, pc, pr, first=(k == 1),
                                last=(k == len(order)))
                pend = built

    nc.compile()
    return nc


def _prep_inputs(x, adj, Wf, bf_, Ww, bw):
    b = ml_dtypes.bfloat16
    x64 = x.astype(np.float64)
    ys = (x64 @ Wf[:, :DI].astype(np.float64).T).astype(np.float32)   # [N, 64]
    u = (x64 @ Wf[:, DI:].astype(np.float64).T + bf_).astype(np.float32)
    asrc = (x64 @ Ww[0, :DI].astype(np.float64)).astype(np.float32)   # [N]
    g = np.exp(asrc.astype(np.float64)).astype(np.float32)            # [N]

    # ysjp[jl, 64c+d] = ys[128c+jl, d]
    ysjp = ys.reshape(NCHUNK, 128, DO).transpose(1, 0, 2).reshape(128, -1)
    # e'^T[j, i] = g[j] * (adj[i, j] > 0), chunk-packed:
    # etp[jl, 128c+il] = e'^T[128c+jl, il]
    mask_t = (adj > 0).T.astype(np.float32)          # [j, i]
    et_full = mask_t * g[:, None]                    # [j, i]
    sfull = et_full.sum(axis=0)                      # [i] row sums (denom)

    in_maps = []
    for c in range(N_CORES):
        blk = slice(ROWS * c, ROWS * (c + 1))
        et = et_full[:, blk]                          # [1024, 128]
        etp = et.reshape(NCHUNK, 128, ROWS).transpose(1, 0, 2).reshape(128, -1)
        m = dict(
            ysjp=np.ascontiguousarray(ysjp).astype(b),
            etp=np.ascontiguousarray(etp).astype(b),
            rinv=np.ascontiguousarray(
                (1.0 / sfull[blk]).reshape(128, 1)).astype(np.float32),
            uflat=np.ascontiguousarray(u[blk].reshape(F_FULL)).astype(b),
        )
        in_maps.append(m)
    return in_maps


def get_program():
    if "nc" not in _CACHE:
        _CACHE["nc"] = _build_program()
    return _CACHE["nc"]


def unpack_output(res_list):
    p_idx = np.arange(128)
    col0 = (p_idx % 32) * DO
    cols = col0[:, None] + np.arange(DO)[None, :]
    out = np.empty((N, DO), np.float32)
    for c in range(N_CORES):
        t = res_list[c]["o"].astype(np.float32)      # [128, 2048]
        out[ROWS * c:ROWS * (c + 1)] = t[p_idx[:, None], cols]
    return out


def kernel(x, adj, Wf, bf, Ww, bw):
    x = np.asarray(x, dtype=np.float32)
    adj = np.asarray(adj, dtype=np.int32)
    Wf = np.asarray(Wf, dtype=np.float32)
    bf_ = np.asarray(bf, dtype=np.float32)
    Ww = np.asarray(Ww, dtype=np.float32)
    bw = np.asarray(bw, dtype=np.float32)
    assert x.shape == (N, DI) and adj.shape == (N, N)

    nc = get_program()
    in_maps = _prep_inputs(x, adj, Wf, bf_, Ww, bw)
    res = run_bass_kernel_spmd(nc, in_maps, core_ids=list(range(N_CORES)))
    return unpack_output(res.results)
